# revision 30
# baseline (speedup 1.0000x reference)
"""Multi-head attention (AttnProcessor2_0) on 8 TRN2 NeuronCores.

Problem: B=2, S=4096, C=640, H=10, Dh=64.
  q/k/v = hs @ W{q,k,v}.T ; per-head scores = q k^T / 8 ; softmax ;
  out = probs v ; y = out @ Wo.T + b_out + hs

Sharding (no collectives): core c -> batch b=c//4, query block g=c%4
(1024 queries).  Each core recomputes full K/V for its batch (head-dim
on partitions), computes its own S/4 x S attention block, output
projection, bias+residual.  Host passes hidden states TRANSPOSED and
ROLLED by the query offset so the same SPMD program works on every
core (softmax+PV are permutation-invariant along the key axis).

Device layout (everything feature-on-partition, token-on-free):
  kT [640, 4096] (5 chunks of 128 = 2 heads each)  "scoresT" = K Q^T
  v  [4096, 650] (65-stride per head: 64 cols + ones col -> softmax
     denominators fall out of the PV matmul as PSUM row 64)
  probs: scoresT in PSUM -> ScalarE exp -> bf16 SBUF (ScalarE is the
     bottleneck engine: S*S*BH/8 = 41.9M exps/core)
  normalization: reciprocal of denom row, rank-1 PE outer product to
     broadcast across partitions (DVE cannot cross partitions), DVE mult.
All matmuls bf16 (f32 PSUM accumulation).
"""

import sys

if "/opt/trn_rl_repo" not in sys.path:
    sys.path.insert(0, "/opt/trn_rl_repo")

from contextlib import ExitStack

import ml_dtypes
import numpy as np

import concourse.bass as bass
import concourse.tile as tile
from concourse import mybir
from concourse.bass import ts

BF16 = mybir.dt.bfloat16
F32 = mybir.dt.float32

B, S, C = 2, 4096, 640
H, DH = 10, 64
NCORES = 8
GROUP = 4  # cores per batch element
SQ = S // GROUP  # 1024 queries per core
SCALE = 0.125  # 1/sqrt(64)
CCH = C // 128  # 5 feature chunks (2 heads each)
NJT = S // 512  # 8 key tiles for K proj
NJC = S // 128  # 32 key chunks for attention
NIT = SQ // 512  # 2 query tiles
VST = DH + 1  # 65: per-head stride in v tiles (ones col appended)

# exp group structure: j-chunks per ScalarE activation call
EXP_GROUPS = [list(range(g, min(g + 3, NJC))) for g in range(0, NJC, 3)]


def build_nc() -> bass.Bass:
    nc = bass.Bass()
    hsT = nc.declare_dram_parameter("hsT", [C, S], BF16, isOutput=False)
    res = nc.declare_dram_parameter("res", [C, SQ], F32, isOutput=False)
    wqT = nc.declare_dram_parameter("wqT", [C, C], BF16, isOutput=False)
    wkT = nc.declare_dram_parameter("wkT", [C, C], BF16, isOutput=False)
    wvT = nc.declare_dram_parameter("wvT", [C, C], BF16, isOutput=False)
    woT = nc.declare_dram_parameter("woT", [C, C], BF16, isOutput=False)
    out = nc.declare_dram_parameter("out", [C, SQ], F32, isOutput=True)

    with ExitStack() as ctx:
        tc = ctx.enter_context(tile.TileContext(nc))
        # outer pool: tensors whose lifetime spans projections AND attention
        sb = ctx.enter_context(tc.tile_pool(name="sb", bufs=1))

        kT_sb = [sb.tile([128, S], BF16, tag=f"kT{i}", name=f"kT{i}") for i in range(CCH)]
        qT_sb = [sb.tile([128, SQ], BF16, tag=f"qT{i}", name=f"qT{i}")
                 for i in range(CCH)]
        v_sb = [sb.tile([128, H * VST], BF16, tag=f"v{j}", name=f"v{j}") for j in range(NJC)]
        ones_sb = sb.tile([128, DH], BF16, tag="ones", name="ones")
        nc.vector.memset(ones_sb[:], 1.0)
        wo_sb = []
        for h in range(H):
            w = sb.tile([128, C], BF16, tag=f"wo{h}", name=f"wo{h}")
            nc.vector.memset(w[DH:128, :], 0.0)
            nc.sync.dma_start(w[0:DH, :], woT[ts(h, DH), :])
            wo_sb.append(w)

        # ---------------- load + projections phase ----------------
        with tc.tile_pool(name="load", bufs=1) as load, \
             tc.tile_pool(name="pp", bufs=3, space="PSUM") as pp:
            hsT_sb = []
            wq_sb, wk_sb, wv_sb = [], [], []
            for i in range(CCH):
                t = load.tile([128, S], BF16, tag=f"hsT{i}", name=f"hsT{i}")
                nc.sync.dma_start(t[:], hsT[ts(i, 128), :])
                hsT_sb.append(t)
                for name, lst, src in (("wq", wq_sb, wqT), ("wk", wk_sb, wkT),
                                       ("wv", wv_sb, wvT)):
                    w = load.tile([128, C], BF16, tag=f"{name}{i}", name=f"{name}{i}")
                    nc.sync.dma_start(w[:], src[ts(i, 128), :])
                    lst.append(w)

            # K projection: kT[d, j]
            for dc in range(CCH):
                for jt in range(NJT):
                    ps = pp.tile([128, 512], F32, tag="pp", name="pp")
                    for cc in range(CCH):
                        nc.tensor.matmul(
                            ps[:],
                            wk_sb[cc][:, ts(dc, 128)],
                            hsT_sb[cc][:, ts(jt, 512)],
                            start=(cc == 0),
                            stop=(cc == CCH - 1),
                        )
                    nc.vector.tensor_copy(kT_sb[dc][:, ts(jt, 512)], ps[:])

            # Q projection: qT[d, i]
            for dc in range(CCH):
                for it in range(NIT):
                    ps = pp.tile([128, 512], F32, tag="pp", name="pp")
                    for cc in range(CCH):
                        nc.tensor.matmul(
                            ps[:],
                            wq_sb[cc][:, ts(dc, 128)],
                            hsT_sb[cc][:, ts(it, 512)],
                            start=(cc == 0),
                            stop=(cc == CCH - 1),
                        )
                    nc.vector.tensor_copy(qT_sb[dc][:, ts(it, 512)], ps[:])

            # V projection: v[j, h*65 + dh] + ones cols
            for jc in range(NJC):
                vt = v_sb[jc]
                v3 = vt[:].rearrange("p (h x) -> p h x", x=VST)
                nc.vector.memset(v3[:, :, DH : DH + 1], 1.0)
                for d0, dn in ((0, 512), (512, 128)):
                    ps = pp.tile([128, dn], F32, tag="pp", name="pp")
                    for cc in range(CCH):
                        nc.tensor.matmul(
                            ps[:],
                            hsT_sb[cc][:, ts(jc, 128)],
                            wv_sb[cc][:, d0 : d0 + dn],
                            start=(cc == 0),
                            stop=(cc == CCH - 1),
                        )
                    nc.vector.tensor_copy(
                        v3[:, d0 // DH : (d0 + dn) // DH, 0:DH],
                        ps[:].rearrange("p (h x) -> p h x", x=DH),
                    )

        # ---------------- attention phase ----------------
        attn_sb = [sb.tile([128, SQ], BF16, tag=f"attn{h}", name=f"attn{h}")
                   for h in range(H)]
        for h in range(H):
            nc.vector.memset(attn_sb[h][DH:128, :], 0.0)
        with tc.tile_pool(name="ap", bufs=1, space="PSUM") as ap, \
             tc.tile_pool(name="pt", bufs=4) as pt_pool, \
             tc.tile_pool(name="scratch", bufs=4) as scratch:
            def norm_dve(h, pv, p_isl):
                # phase 1 (DVE only, emitted right after the head's last PV):
                # reciprocal of the denominator row + copy the unnormalized
                # output out of PSUM.  No PE instructions -> no PE stall.
                recip = scratch.tile([DH + 1, 512], BF16, tag="recip",
                                     name="recip")
                with nc.allow_low_precision(reason="softmax recip bf16"):
                    nc.vector.reciprocal(recip[DH : DH + 1, :],
                                         pv[DH : DH + 1, :])
                raw = scratch.tile([DH, 512], BF16, tag="raw", name="raw")
                nc.vector.tensor_copy(raw[:], pv[0:DH, :])
                return (h, pv, p_isl, recip, raw)

            def norm_pe(states, anchor):
                # phase 2 (emitted during the NEXT pair's exp-group 2, when
                # the ~3.4us DVE reciprocals have finished): rank-1 PE outer
                # products broadcast the reciprocals across partitions
                # reusing the pv banks in place, then DVE multiplies release
                # the pv slots.  The explicit dep pins the R matmuls behind
                # the anchor QK -- Tile's cost model underprices
                # InstReciprocal and otherwise schedules them early.
                r_mms = []
                for h, pv, p_isl, recip, raw in states:
                    r_mm = nc.tensor.matmul(
                        pv[0:DH, :],
                        ones_sb[DH : DH + 1, :],
                        recip[DH : DH + 1, :],
                        start=True,
                        stop=True,
                    )
                    if anchor is not None:
                        tile.add_dep_helper(
                            r_mm.ins, anchor.ins, sync=False,
                            reason="norm R after anchor QK (hide recip)",
                        )
                    r_mms.append(r_mm)
                for h, pv, p_isl, recip, raw in states:
                    nc.vector.tensor_mul(
                        attn_sb[h][0:DH, p_isl], raw[:], pv[0:DH, :]
                    )
                return r_mms

            # QK uses K=64 (the head dim) with the two heads of a chunk on
            # disjoint PE row groups (base partitions 0 / 64): alternating
            # A/B emission runs them concurrently on the array, halving QK
            # time vs a full-K instruction.  The concurrent PV stream (K=128)
            # keeps the HAM activity monitor warm.  The first pair's-worth of
            # PV groups after a head-pair boundary is deferred until the
            # previous pair's norm matmuls are in the PE stream, so the
            # 2-slot pv rotation can never deadlock the in-order PE queue.
            pending = None
            for it in range(NIT):
                isl = ts(it, 512)
                for hp in range(CCH):
                    pvs = [
                        ap.tile([DH + 1, 512], F32, tag="pv", bufs=2, name="pv")
                        for _ in range(2)
                    ]
                    pts = {}
                    deferred = pending is not None

                    def emit_pv(h01, gi, first_dep=None):
                        first = first_dep is not None
                        for k, jc in enumerate(EXP_GROUPS[gi]):
                            mm = nc.tensor.matmul(
                                pvs[h01][:],
                                v_sb[jc][:, (2 * hp + h01) * VST
                                         : (2 * hp + h01 + 1) * VST],
                                pts[(h01, gi)][:, ts(k, 512)],
                                start=(jc == 0),
                                stop=(jc == NJC - 1),
                            )
                            if first:
                                tile.add_dep_helper(
                                    mm.ins, first_dep.ins, sync=False,
                                    reason="deferred PV after norm R",
                                )
                                first = False

                    for gi, g in enumerate(EXP_GROUPS):
                        scs = [
                            ap.tile([128, 512 * len(g)], F32, tag="sc",
                                    bufs=2, name="sc")
                            for _ in range(2)
                        ]
                        last_qk = None
                        for k, jc in enumerate(g):
                            for h01 in range(2):
                                po = h01 * DH
                                last_qk = nc.tensor.matmul(
                                    scs[h01][:, ts(k, 512)],
                                    kT_sb[hp][po : po + DH, ts(jc, 128)],
                                    qT_sb[hp][po : po + DH, isl],
                                    start=True,
                                    stop=True,
                                )
                        for h01 in range(2):
                            pt = pt_pool.tile([128, 512 * len(g)], BF16,
                                              tag="pt", bufs=8, name="pt")
                            nc.scalar.activation(
                                pt[:], scs[h01][:],
                                mybir.ActivationFunctionType.Exp,
                                bias=0.0, scale=SCALE,
                            )
                            pts[(h01, gi)] = pt
                        if deferred:
                            if gi < 2:
                                continue
                            if gi == 2:
                                r_mms = norm_pe(pending, last_qk)
                                pending = None
                                deferred = False
                                for h01 in range(2):
                                    for gj in range(3):
                                        emit_pv(h01, gj,
                                                r_mms[h01] if gj == 0 else None)
                                continue
                        for h01 in range(2):
                            emit_pv(h01, gi)
                    pending = (
                        norm_dve(2 * hp, pvs[0], isl),
                        norm_dve(2 * hp + 1, pvs[1], isl),
                    )
            norm_pe(pending, None)

        # ---------------- output projection + bias/residual ----------------
        with tc.tile_pool(name="op", bufs=1, space="PSUM") as op, \
             tc.tile_pool(name="ob", bufs=3) as ob:
            for ec in range(CCH):
                for it in range(NIT):
                    ps = op.tile([128, 512], F32, tag="yo", bufs=2, name="yo")
                    for h in range(H):
                        nc.tensor.matmul(
                            ps[:],
                            wo_sb[h][:, ts(ec, 128)],
                            attn_sb[h][:, ts(it, 512)],
                            start=(h == 0),
                            stop=(h == H - 1),
                        )
                    rt = ob.tile([128, 512], F32, tag="rt", name="rt")
                    nc.sync.dma_start(rt[:], res[ts(ec, 128), ts(it, 512)])
                    ot = ob.tile([128, 512], F32, tag="ot", name="ot")
                    nc.vector.tensor_add(ot[:], ps[:], rt[:])
                    nc.sync.dma_start(out[ts(ec, 128), ts(it, 512)], ot[:])

    _spill_matmul_waits(nc)
    return nc


# walrus embedded-sync-wait capacity per BIR opcode.  Matmult holds a
# single wait; excess waits hoist onto the paired Ldweights (in-order
# issue on PE makes that equivalent).  Other compute ops spill onto
# EventSemaphore carrier instructions inserted just before them on the
# same engine.  DMACopy / Drain / EventSemaphore handle many waits
# natively (bacc emits such itself) and are left alone.
_WAIT_CAPS = {
    "InstMatmult": 1,
    "InstLdweights": 1,
    "InstActivation": 1,
    "InstReciprocal": 1,
    "InstTensorTensor": 1,
    "InstTensorCopy": 1,
    "InstTensorScalarPtr": 1,
    "InstTensorReduce": 1,
    "InstMemset": 1,
    "InstDMACopy": 1,
    "InstDrain": 1,
    "InstCustomDveAnt": 1,
}
_ES_CAP = 2  # waits per EventSemaphore carrier (walrus: <=2 waits, <=1 update)


def _spill_matmul_waits(nc: bass.Bass) -> None:
    spill_id = [0]

    def carriers(excess, engine):
        out = []
        for i in range(0, len(excess), _ES_CAP):
            es = mybir.InstEventSemaphore(
                name=f"wait-spill-{spill_id[0]}", ins=[], outs=[]
            )
            spill_id[0] += 1
            es.engine = engine
            es.sync_info = mybir.SyncInfo(
                on_wait=excess[i : i + _ES_CAP], on_update=[]
            )
            out.append(es)
        return out

    for f in nc.m.functions:
        for blk in f.blocks:
            insts = blk.instructions
            i = 0
            while i < len(insts):
                inst = insts[i]
                tn = type(inst).__name__
                cap = _WAIT_CAPS.get(tn)
                si = inst.sync_info
                if cap is None or si is None or len(si.on_wait) <= cap:
                    i += 1
                    continue
                w = list(si.on_wait)
                keep, excess = w[-cap:], w[:-cap]
                prev = insts[i - 1] if i > 0 else None
                if (
                    tn == "InstMatmult"
                    and prev is not None
                    and type(prev).__name__ == "InstLdweights"
                    and len(((prev.sync_info and prev.sync_info.on_wait) or []))
                    + len(excess) <= 1
                ):
                    psi = prev.sync_info
                    pw = list(psi.on_wait) if psi is not None else []
                    pu = list(psi.on_update) if psi is not None else []
                    prev.sync_info = mybir.SyncInfo(on_wait=pw + excess, on_update=pu)
                else:
                    new = carriers(excess, inst.engine)
                    insts[i:i] = new
                    i += len(new)
                inst.sync_info = mybir.SyncInfo(
                    on_wait=keep, on_update=list(si.on_update)
                )
                i += 1


_CACHED_NC = None


def get_nc() -> bass.Bass:
    global _CACHED_NC
    if _CACHED_NC is None:
        _CACHED_NC = build_nc()
    return _CACHED_NC


def make_in_maps(hidden_states, Wq, Wk, Wv, Wo, b_out):
    hs = np.asarray(hidden_states, dtype=np.float32)
    bf = ml_dtypes.bfloat16
    wqT = np.ascontiguousarray(np.asarray(Wq, np.float32).T).astype(bf)
    wkT = np.ascontiguousarray(np.asarray(Wk, np.float32).T).astype(bf)
    wvT = np.ascontiguousarray(np.asarray(Wv, np.float32).T).astype(bf)
    woT = np.ascontiguousarray(np.asarray(Wo, np.float32).T).astype(bf)
    bias = np.asarray(b_out, np.float32).reshape(C, 1)
    in_maps = []
    for c in range(NCORES):
        b, g = divmod(c, GROUP)
        i0 = g * SQ
        hsTb = hs[b].T  # [C, S]
        in_maps.append(
            {
                "hsT": np.ascontiguousarray(np.roll(hsTb, -i0, axis=1)).astype(bf),
                "res": np.ascontiguousarray(hsTb[:, i0 : i0 + SQ]) + bias,
                "wqT": wqT,
                "wkT": wkT,
                "wvT": wvT,
                "woT": woT,
            }
        )
    return in_maps


def assemble(results) -> np.ndarray:
    y = np.empty((B, S, C), np.float32)
    for c in range(NCORES):
        b, g = divmod(c, GROUP)
        i0 = g * SQ
        y[b, i0 : i0 + SQ, :] = np.asarray(results[c]["out"], np.float32).T
    return y


def kernel(**inputs) -> np.ndarray:
    from concourse.bass_utils import run_bass_kernel_spmd

    nc = get_nc()
    in_maps = make_in_maps(**inputs)
    res = run_bass_kernel_spmd(nc, in_maps, list(range(NCORES)))
    return assemble(res.results)


if __name__ == "__main__":
    import reference

    inputs = {k: np.asarray(v) for k, v in reference.setup_inputs().items()}
    got = kernel(**inputs)
    want = np.asarray(reference.reference(**inputs))
    err = np.linalg.norm(got - want) / np.linalg.norm(want)
    print("Relative error:", err)


# revision 31
# speedup vs baseline: 1.1622x; 1.1622x over previous
"""Multi-head attention (AttnProcessor2_0) on 8 TRN2 NeuronCores.

Problem: B=2, S=4096, C=640, H=10, Dh=64.
  q/k/v = hs @ W{q,k,v}.T ; per-head scores = q k^T / 8 ; softmax ;
  out = probs v ; y = out @ Wo.T + b_out + hs

Sharding (no collectives): core c -> batch b=c//4, query block g=c%4
(1024 queries).  Each core recomputes full K/V for its batch (head-dim
on partitions), computes its own S/4 x S attention block, output
projection, bias+residual.  Host passes hidden states TRANSPOSED and
ROLLED by the query offset so the same SPMD program works on every
core (softmax+PV are permutation-invariant along the key axis).

Device layout (everything feature-on-partition, token-on-free):
  kT [640, 4096] (5 chunks of 128 = 2 heads each)  "scoresT" = K Q^T
  v  [4096, 650] (65-stride per head: 64 cols + ones col -> softmax
     denominators fall out of the PV matmul as PSUM row 64)
  probs: scoresT in PSUM -> ScalarE exp -> bf16 SBUF (ScalarE is the
     bottleneck engine: S*S*BH/8 = 41.9M exps/core)
  normalization: reciprocal of denom row, rank-1 PE outer product to
     broadcast across partitions (DVE cannot cross partitions), DVE mult.
All matmuls bf16 (f32 PSUM accumulation).
"""

import sys

if "/opt/trn_rl_repo" not in sys.path:
    sys.path.insert(0, "/opt/trn_rl_repo")

from contextlib import ExitStack

import ml_dtypes
import numpy as np

import concourse.bass as bass
import concourse.tile as tile
from concourse import mybir
from concourse.bass import ts

BF16 = mybir.dt.bfloat16
F32 = mybir.dt.float32

B, S, C = 2, 4096, 640
H, DH = 10, 64
NCORES = 8
GROUP = 4  # cores per batch element
SQ = S // GROUP  # 1024 queries per core
SCALE = 0.125  # 1/sqrt(64)
CCH = C // 128  # 5 feature chunks (2 heads each)
NJT = S // 512  # 8 key tiles for K proj
NJC = S // 128  # 32 key chunks for attention
NIT = SQ // 512  # 2 query tiles
VST = DH + 1  # 65: per-head stride in v tiles (ones col appended)

# exp group structure: j-chunks per ScalarE activation call
EXP_GROUPS = [list(range(g, min(g + 3, NJC))) for g in range(0, NJC, 3)]


def build_nc() -> bass.Bass:
    nc = bass.Bass()
    hsT = nc.declare_dram_parameter("hsT", [C, S], BF16, isOutput=False)
    res = nc.declare_dram_parameter("res", [C, SQ], F32, isOutput=False)
    wqT = nc.declare_dram_parameter("wqT", [C, C], BF16, isOutput=False)
    wkT = nc.declare_dram_parameter("wkT", [C, C], BF16, isOutput=False)
    wvT = nc.declare_dram_parameter("wvT", [C, C], BF16, isOutput=False)
    woT = nc.declare_dram_parameter("woT", [C, C], BF16, isOutput=False)
    out = nc.declare_dram_parameter("out", [C, SQ], F32, isOutput=True)

    with ExitStack() as ctx:
        tc = ctx.enter_context(tile.TileContext(nc))
        # outer pool: tensors whose lifetime spans projections AND attention
        sb = ctx.enter_context(tc.tile_pool(name="sb", bufs=1))

        kT_sb = [sb.tile([128, S], BF16, tag=f"kT{i}", name=f"kT{i}") for i in range(CCH)]
        # per-head q, zero-padded to full 128-row contraction: partial-K
        # (K=64) matmuls keep the PE HAM-throttled at 1.2 GHz -- padding the
        # contraction with zero rows is exact and runs at the warm rate.
        qTz_sb = [
            [sb.tile([128, SQ], BF16, tag=f"qz{i}_{p}", name=f"qz{i}_{p}")
             for p in range(2)]
            for i in range(CCH)
        ]
        v_sb = [sb.tile([128, H * VST], BF16, tag=f"v{j}", name=f"v{j}") for j in range(NJC)]
        ones_sb = sb.tile([128, DH], BF16, tag="ones", name="ones")
        nc.vector.memset(ones_sb[:], 1.0)
        wo_sb = []
        for h in range(H):
            w = sb.tile([128, C], BF16, tag=f"wo{h}", name=f"wo{h}")
            nc.vector.memset(w[DH:128, :], 0.0)
            nc.sync.dma_start(w[0:DH, :], woT[ts(h, DH), :])
            wo_sb.append(w)

        # ---------------- load + projections phase ----------------
        with tc.tile_pool(name="load", bufs=1) as load, \
             tc.tile_pool(name="pp", bufs=3, space="PSUM") as pp:
            hsT_sb = []
            wq_sb, wk_sb, wv_sb = [], [], []
            for i in range(CCH):
                t = load.tile([128, S], BF16, tag=f"hsT{i}", name=f"hsT{i}")
                nc.sync.dma_start(t[:], hsT[ts(i, 128), :])
                hsT_sb.append(t)
                for name, lst, src in (("wq", wq_sb, wqT), ("wk", wk_sb, wkT),
                                       ("wv", wv_sb, wvT)):
                    w = load.tile([128, C], BF16, tag=f"{name}{i}", name=f"{name}{i}")
                    nc.sync.dma_start(w[:], src[ts(i, 128), :])
                    lst.append(w)

            # K projection: kT[d, j]
            for dc in range(CCH):
                for jt in range(NJT):
                    ps = pp.tile([128, 512], F32, tag="pp", name="pp")
                    for cc in range(CCH):
                        nc.tensor.matmul(
                            ps[:],
                            wk_sb[cc][:, ts(dc, 128)],
                            hsT_sb[cc][:, ts(jt, 512)],
                            start=(cc == 0),
                            stop=(cc == CCH - 1),
                        )
                    nc.vector.tensor_copy(kT_sb[dc][:, ts(jt, 512)], ps[:])

            # Q projection: per-head rows of qT, zero-padded to K=128
            for dc in range(CCH):
                nc.vector.memset(qTz_sb[dc][0][DH:128, :], 0.0)
                nc.vector.memset(qTz_sb[dc][1][0:DH, :], 0.0)
                for it in range(NIT):
                    ps = pp.tile([128, 512], F32, tag="pp", name="pp")
                    for cc in range(CCH):
                        nc.tensor.matmul(
                            ps[:],
                            wq_sb[cc][:, ts(dc, 128)],
                            hsT_sb[cc][:, ts(it, 512)],
                            start=(cc == 0),
                            stop=(cc == CCH - 1),
                        )
                    nc.vector.tensor_copy(
                        qTz_sb[dc][0][0:DH, ts(it, 512)], ps[0:DH, :])
                    nc.vector.tensor_copy(
                        qTz_sb[dc][1][DH:128, ts(it, 512)], ps[DH:128, :])

            # V projection: v[j, h*65 + dh] + ones cols
            for jc in range(NJC):
                vt = v_sb[jc]
                v3 = vt[:].rearrange("p (h x) -> p h x", x=VST)
                nc.vector.memset(v3[:, :, DH : DH + 1], 1.0)
                for d0, dn in ((0, 512), (512, 128)):
                    ps = pp.tile([128, dn], F32, tag="pp", name="pp")
                    for cc in range(CCH):
                        nc.tensor.matmul(
                            ps[:],
                            hsT_sb[cc][:, ts(jc, 128)],
                            wv_sb[cc][:, d0 : d0 + dn],
                            start=(cc == 0),
                            stop=(cc == CCH - 1),
                        )
                    nc.vector.tensor_copy(
                        v3[:, d0 // DH : (d0 + dn) // DH, 0:DH],
                        ps[:].rearrange("p (h x) -> p h x", x=DH),
                    )

        # ---------------- attention phase ----------------
        attn_sb = [sb.tile([128, SQ], BF16, tag=f"attn{h}", name=f"attn{h}")
                   for h in range(H)]
        for h in range(H):
            nc.vector.memset(attn_sb[h][DH:128, :], 0.0)
        with tc.tile_pool(name="ap", bufs=1, space="PSUM") as ap, \
             tc.tile_pool(name="pt", bufs=4) as pt_pool, \
             tc.tile_pool(name="scratch", bufs=4) as scratch:
            def norm_dve(h, pv, p_isl):
                # phase 1 (DVE only, emitted right after the head's last PV):
                # reciprocal of the denominator row + copy the unnormalized
                # output out of PSUM.  No PE instructions -> no PE stall.
                recip = scratch.tile([DH + 1, 512], BF16, tag="recip",
                                     name="recip")
                with nc.allow_low_precision(reason="softmax recip bf16"):
                    nc.vector.reciprocal(recip[DH : DH + 1, :],
                                         pv[DH : DH + 1, :])
                raw = scratch.tile([DH, 512], BF16, tag="raw", name="raw")
                nc.vector.tensor_copy(raw[:], pv[0:DH, :])
                return (h, pv, p_isl, recip, raw)

            def norm_pe(state, anchor):
                # phase 2 (emitted during the NEXT head's exp-group 1, when
                # the DVE chain has long finished): rank-1 PE outer product
                # broadcasts the reciprocal across partitions reusing the pv
                # bank in place, then one DVE multiply + slot release.
                h, pv, p_isl, recip, raw = state
                r_mm = nc.tensor.matmul(
                    pv[0:DH, :],
                    ones_sb[DH : DH + 1, :],
                    recip[DH : DH + 1, :],
                    start=True,
                    stop=True,
                )
                if anchor is not None:
                    tile.add_dep_helper(
                        r_mm.ins, anchor.ins, sync=False,
                        reason="norm R after anchor QK (hide recip latency)",
                    )
                nc.vector.tensor_mul(
                    attn_sb[h][0:DH, p_isl], raw[:], pv[0:DH, :]
                )

            pending = None
            for it in range(NIT):
                isl = ts(it, 512)
                for hp in range(CCH):
                    for h in (2 * hp, 2 * hp + 1):
                        pv = ap.tile([DH + 1, 512], F32, tag="pv", bufs=2, name="pv")
                        for gi, g in enumerate(EXP_GROUPS):
                            sc = ap.tile([128, 512 * len(g)], F32, tag="sc",
                                         bufs=2, name="sc")
                            last_qk = None
                            for k, jc in enumerate(g):
                                last_qk = nc.tensor.matmul(
                                    sc[:, ts(k, 512)],
                                    kT_sb[hp][:, ts(jc, 128)],
                                    qTz_sb[hp][h % 2][:, isl],
                                    start=True,
                                    stop=True,
                                )
                            pt = pt_pool.tile([128, 512 * len(g)], BF16, tag="pt",
                                              name="pt")
                            nc.scalar.activation(
                                pt[:], sc[:], mybir.ActivationFunctionType.Exp,
                                bias=0.0, scale=SCALE,
                            )
                            if pending is not None and gi == 3:
                                norm_pe(pending, last_qk)
                                pending = None
                            for k, jc in enumerate(g):
                                nc.tensor.matmul(
                                    pv[:],
                                    v_sb[jc][:, h * VST : (h + 1) * VST],
                                    pt[:, ts(k, 512)],
                                    start=(jc == 0),
                                    stop=(jc == NJC - 1),
                                )
                        pending = norm_dve(h, pv, isl)
            norm_pe(pending, None)

        # ---------------- output projection + bias/residual ----------------
        with tc.tile_pool(name="op", bufs=1, space="PSUM") as op, \
             tc.tile_pool(name="ob", bufs=3) as ob:
            for ec in range(CCH):
                for it in range(NIT):
                    ps = op.tile([128, 512], F32, tag="yo", bufs=2, name="yo")
                    for h in range(H):
                        nc.tensor.matmul(
                            ps[:],
                            wo_sb[h][:, ts(ec, 128)],
                            attn_sb[h][:, ts(it, 512)],
                            start=(h == 0),
                            stop=(h == H - 1),
                        )
                    rt = ob.tile([128, 512], F32, tag="rt", name="rt")
                    nc.sync.dma_start(rt[:], res[ts(ec, 128), ts(it, 512)])
                    ot = ob.tile([128, 512], F32, tag="ot", name="ot")
                    nc.vector.tensor_add(ot[:], ps[:], rt[:])
                    nc.sync.dma_start(out[ts(ec, 128), ts(it, 512)], ot[:])

    _spill_matmul_waits(nc)
    return nc


# walrus embedded-sync-wait capacity per BIR opcode.  Matmult holds a
# single wait; excess waits hoist onto the paired Ldweights (in-order
# issue on PE makes that equivalent).  Other compute ops spill onto
# EventSemaphore carrier instructions inserted just before them on the
# same engine.  DMACopy / Drain / EventSemaphore handle many waits
# natively (bacc emits such itself) and are left alone.
_WAIT_CAPS = {
    "InstMatmult": 1,
    "InstLdweights": 1,
    "InstActivation": 1,
    "InstReciprocal": 1,
    "InstTensorTensor": 1,
    "InstTensorCopy": 1,
    "InstTensorScalarPtr": 1,
    "InstTensorReduce": 1,
    "InstMemset": 1,
    "InstDMACopy": 1,
    "InstDrain": 1,
    "InstCustomDveAnt": 1,
}
_ES_CAP = 2  # waits per EventSemaphore carrier (walrus: <=2 waits, <=1 update)


def _spill_matmul_waits(nc: bass.Bass) -> None:
    spill_id = [0]

    def carriers(excess, engine):
        out = []
        for i in range(0, len(excess), _ES_CAP):
            es = mybir.InstEventSemaphore(
                name=f"wait-spill-{spill_id[0]}", ins=[], outs=[]
            )
            spill_id[0] += 1
            es.engine = engine
            es.sync_info = mybir.SyncInfo(
                on_wait=excess[i : i + _ES_CAP], on_update=[]
            )
            out.append(es)
        return out

    for f in nc.m.functions:
        for blk in f.blocks:
            insts = blk.instructions
            i = 0
            while i < len(insts):
                inst = insts[i]
                tn = type(inst).__name__
                cap = _WAIT_CAPS.get(tn)
                si = inst.sync_info
                if cap is None or si is None or len(si.on_wait) <= cap:
                    i += 1
                    continue
                w = list(si.on_wait)
                keep, excess = w[-cap:], w[:-cap]
                prev = insts[i - 1] if i > 0 else None
                if (
                    tn == "InstMatmult"
                    and prev is not None
                    and type(prev).__name__ == "InstLdweights"
                    and len(((prev.sync_info and prev.sync_info.on_wait) or []))
                    + len(excess) <= 1
                ):
                    psi = prev.sync_info
                    pw = list(psi.on_wait) if psi is not None else []
                    pu = list(psi.on_update) if psi is not None else []
                    prev.sync_info = mybir.SyncInfo(on_wait=pw + excess, on_update=pu)
                else:
                    new = carriers(excess, inst.engine)
                    insts[i:i] = new
                    i += len(new)
                inst.sync_info = mybir.SyncInfo(
                    on_wait=keep, on_update=list(si.on_update)
                )
                i += 1


_CACHED_NC = None


def get_nc() -> bass.Bass:
    global _CACHED_NC
    if _CACHED_NC is None:
        _CACHED_NC = build_nc()
    return _CACHED_NC


def make_in_maps(hidden_states, Wq, Wk, Wv, Wo, b_out):
    hs = np.asarray(hidden_states, dtype=np.float32)
    bf = ml_dtypes.bfloat16
    wqT = np.ascontiguousarray(np.asarray(Wq, np.float32).T).astype(bf)
    wkT = np.ascontiguousarray(np.asarray(Wk, np.float32).T).astype(bf)
    wvT = np.ascontiguousarray(np.asarray(Wv, np.float32).T).astype(bf)
    woT = np.ascontiguousarray(np.asarray(Wo, np.float32).T).astype(bf)
    bias = np.asarray(b_out, np.float32).reshape(C, 1)
    in_maps = []
    for c in range(NCORES):
        b, g = divmod(c, GROUP)
        i0 = g * SQ
        hsTb = hs[b].T  # [C, S]
        in_maps.append(
            {
                "hsT": np.ascontiguousarray(np.roll(hsTb, -i0, axis=1)).astype(bf),
                "res": np.ascontiguousarray(hsTb[:, i0 : i0 + SQ]) + bias,
                "wqT": wqT,
                "wkT": wkT,
                "wvT": wvT,
                "woT": woT,
            }
        )
    return in_maps


def assemble(results) -> np.ndarray:
    y = np.empty((B, S, C), np.float32)
    for c in range(NCORES):
        b, g = divmod(c, GROUP)
        i0 = g * SQ
        y[b, i0 : i0 + SQ, :] = np.asarray(results[c]["out"], np.float32).T
    return y


def kernel(**inputs) -> np.ndarray:
    from concourse.bass_utils import run_bass_kernel_spmd

    nc = get_nc()
    in_maps = make_in_maps(**inputs)
    res = run_bass_kernel_spmd(nc, in_maps, list(range(NCORES)))
    return assemble(res.results)


if __name__ == "__main__":
    import reference

    inputs = {k: np.asarray(v) for k, v in reference.setup_inputs().items()}
    got = kernel(**inputs)
    want = np.asarray(reference.reference(**inputs))
    err = np.linalg.norm(got - want) / np.linalg.norm(want)
    print("Relative error:", err)


# revision 32
# speedup vs baseline: 1.1716x; 1.0081x over previous
"""Multi-head attention (AttnProcessor2_0) on 8 TRN2 NeuronCores.

Problem: B=2, S=4096, C=640, H=10, Dh=64.
  q/k/v = hs @ W{q,k,v}.T ; per-head scores = q k^T / 8 ; softmax ;
  out = probs v ; y = out @ Wo.T + b_out + hs

Sharding (no collectives): core c -> batch b=c//4, query block g=c%4
(1024 queries).  Each core recomputes full K/V for its batch (head-dim
on partitions), computes its own S/4 x S attention block, output
projection, bias+residual.  Host passes hidden states TRANSPOSED and
ROLLED by the query offset so the same SPMD program works on every
core (softmax+PV are permutation-invariant along the key axis).

Device layout (everything feature-on-partition, token-on-free):
  kT [640, 4096] (5 chunks of 128 = 2 heads each)  "scoresT" = K Q^T
  v  [4096, 650] (65-stride per head: 64 cols + ones col -> softmax
     denominators fall out of the PV matmul as PSUM row 64)
  probs: scoresT in PSUM -> ScalarE exp -> bf16 SBUF (ScalarE is the
     bottleneck engine: S*S*BH/8 = 41.9M exps/core)
  normalization: reciprocal of denom row, rank-1 PE outer product to
     broadcast across partitions (DVE cannot cross partitions), DVE mult.
All matmuls bf16 (f32 PSUM accumulation).
"""

import sys

if "/opt/trn_rl_repo" not in sys.path:
    sys.path.insert(0, "/opt/trn_rl_repo")

from contextlib import ExitStack

import ml_dtypes
import numpy as np

import concourse.bass as bass
import concourse.tile as tile
from concourse import mybir
from concourse.bass import ts

BF16 = mybir.dt.bfloat16
F32 = mybir.dt.float32

B, S, C = 2, 4096, 640
H, DH = 10, 64
NCORES = 8
GROUP = 4  # cores per batch element
SQ = S // GROUP  # 1024 queries per core
SCALE = 0.125  # 1/sqrt(64)
CCH = C // 128  # 5 feature chunks (2 heads each)
NJT = S // 512  # 8 key tiles for K proj
NJC = S // 128  # 32 key chunks for attention
NIT = SQ // 512  # 2 query tiles
VST = DH + 1  # 65: per-head stride in v tiles (ones col appended)

# exp group structure: j-chunks per ScalarE activation call
EXP_GROUPS = [list(range(g, min(g + 3, NJC))) for g in range(0, NJC, 3)]


def build_nc() -> bass.Bass:
    nc = bass.Bass()
    hsT = nc.declare_dram_parameter("hsT", [C, S], BF16, isOutput=False)
    res = nc.declare_dram_parameter("res", [C, SQ], F32, isOutput=False)
    wqT = nc.declare_dram_parameter("wqT", [C, C], BF16, isOutput=False)
    wkT = nc.declare_dram_parameter("wkT", [C, C], BF16, isOutput=False)
    wvT = nc.declare_dram_parameter("wvT", [C, C], BF16, isOutput=False)
    woT = nc.declare_dram_parameter("woT", [C, C], BF16, isOutput=False)
    out = nc.declare_dram_parameter("out", [C, SQ], F32, isOutput=True)

    with ExitStack() as ctx:
        tc = ctx.enter_context(tile.TileContext(nc))
        # outer pool: tensors whose lifetime spans projections AND attention
        sb = ctx.enter_context(tc.tile_pool(name="sb", bufs=1))

        kT_sb = [sb.tile([128, S], BF16, tag=f"kT{i}", name=f"kT{i}") for i in range(CCH)]
        # per-head q, zero-padded to full 128-row contraction: partial-K
        # (K=64) matmuls keep the PE HAM-throttled at 1.2 GHz -- padding the
        # contraction with zero rows is exact and runs at the warm rate.
        qTz_sb = [
            [sb.tile([128, SQ], BF16, tag=f"qz{i}_{p}", name=f"qz{i}_{p}")
             for p in range(2)]
            for i in range(CCH)
        ]
        v_sb = [sb.tile([128, H * VST], BF16, tag=f"v{j}", name=f"v{j}") for j in range(NJC)]
        ones_sb = sb.tile([128, DH], BF16, tag="ones", name="ones")
        nc.vector.memset(ones_sb[:], 1.0)
        wo_sb = []
        for h in range(H):
            w = sb.tile([128, C], BF16, tag=f"wo{h}", name=f"wo{h}")
            nc.vector.memset(w[DH:128, :], 0.0)
            nc.sync.dma_start(w[0:DH, :], woT[ts(h, DH), :])
            wo_sb.append(w)

        # ---------------- load + projections phase ----------------
        with tc.tile_pool(name="load", bufs=1) as load, \
             tc.tile_pool(name="pp", bufs=3, space="PSUM") as pp:
            hsT_sb = []
            wq_sb, wk_sb, wv_sb = [], [], []
            # wk + the first 512 columns of hsT land first so the K
            # projection (jt-outer) starts ~10us earlier; the rest of hsT
            # streams in behind it.
            for i in range(CCH):
                w = load.tile([128, C], BF16, tag=f"wk{i}", name=f"wk{i}")
                nc.sync.dma_start(w[:], wkT[ts(i, 128), :])
                wk_sb.append(w)
                t = load.tile([128, S], BF16, tag=f"hsT{i}", name=f"hsT{i}")
                nc.sync.dma_start(t[:, 0:512], hsT[ts(i, 128), 0:512])
                hsT_sb.append(t)
            for i in range(CCH):
                nc.sync.dma_start(hsT_sb[i][:, 512:S], hsT[ts(i, 128), 512:S])
            for i in range(CCH):
                for name, lst, srcp in (("wq", wq_sb, wqT), ("wv", wv_sb, wvT)):
                    w = load.tile([128, C], BF16, tag=f"{name}{i}", name=f"{name}{i}")
                    nc.sync.dma_start(w[:], srcp[ts(i, 128), :])
                    lst.append(w)

            # K projection: kT[d, j], jt-outer so each new 512-column slab of
            # hsT unlocks a full stripe of work
            for jt in range(NJT):
                for dc in range(CCH):
                    ps = pp.tile([128, 512], F32, tag="pp", name="pp")
                    for cc in range(CCH):
                        nc.tensor.matmul(
                            ps[:],
                            wk_sb[cc][:, ts(dc, 128)],
                            hsT_sb[cc][:, ts(jt, 512)],
                            start=(cc == 0),
                            stop=(cc == CCH - 1),
                        )
                    nc.vector.tensor_copy(kT_sb[dc][:, ts(jt, 512)], ps[:])

            # Q projection: per-head rows of qT, zero-padded to K=128
            for dc in range(CCH):
                nc.vector.memset(qTz_sb[dc][0][DH:128, :], 0.0)
                nc.vector.memset(qTz_sb[dc][1][0:DH, :], 0.0)
                for it in range(NIT):
                    ps = pp.tile([128, 512], F32, tag="pp", name="pp")
                    for cc in range(CCH):
                        nc.tensor.matmul(
                            ps[:],
                            wq_sb[cc][:, ts(dc, 128)],
                            hsT_sb[cc][:, ts(it, 512)],
                            start=(cc == 0),
                            stop=(cc == CCH - 1),
                        )
                    nc.vector.tensor_copy(
                        qTz_sb[dc][0][0:DH, ts(it, 512)], ps[0:DH, :])
                    nc.vector.tensor_copy(
                        qTz_sb[dc][1][DH:128, ts(it, 512)], ps[DH:128, :])

            # V projection: v[j, h*65 + dh] + ones cols
            for jc in range(NJC):
                vt = v_sb[jc]
                v3 = vt[:].rearrange("p (h x) -> p h x", x=VST)
                nc.vector.memset(v3[:, :, DH : DH + 1], 1.0)
                for d0, dn in ((0, 512), (512, 128)):
                    ps = pp.tile([128, dn], F32, tag="pp", name="pp")
                    for cc in range(CCH):
                        nc.tensor.matmul(
                            ps[:],
                            hsT_sb[cc][:, ts(jc, 128)],
                            wv_sb[cc][:, d0 : d0 + dn],
                            start=(cc == 0),
                            stop=(cc == CCH - 1),
                        )
                    nc.vector.tensor_copy(
                        v3[:, d0 // DH : (d0 + dn) // DH, 0:DH],
                        ps[:].rearrange("p (h x) -> p h x", x=DH),
                    )

        # ---------------- attention phase ----------------
        attn_sb = [sb.tile([128, SQ], BF16, tag=f"attn{h}", name=f"attn{h}")
                   for h in range(H)]
        for h in range(H):
            nc.vector.memset(attn_sb[h][DH:128, :], 0.0)
        with tc.tile_pool(name="ap", bufs=1, space="PSUM") as ap, \
             tc.tile_pool(name="pt", bufs=4) as pt_pool, \
             tc.tile_pool(name="scratch", bufs=4) as scratch:
            def norm_dve(h, pv, p_isl):
                # phase 1 (DVE only, emitted right after the head's last PV):
                # reciprocal of the denominator row + copy the unnormalized
                # output out of PSUM.  No PE instructions -> no PE stall.
                recip = scratch.tile([DH + 1, 512], BF16, tag="recip",
                                     name="recip")
                with nc.allow_low_precision(reason="softmax recip bf16"):
                    nc.vector.reciprocal(recip[DH : DH + 1, :],
                                         pv[DH : DH + 1, :])
                raw = scratch.tile([DH, 512], BF16, tag="raw", name="raw")
                nc.vector.tensor_copy(raw[:], pv[0:DH, :])
                return (h, pv, p_isl, recip, raw)

            def norm_pe(state, anchor):
                # phase 2 (emitted during the NEXT head's exp-group 1, when
                # the DVE chain has long finished): rank-1 PE outer product
                # broadcasts the reciprocal across partitions reusing the pv
                # bank in place, then one DVE multiply + slot release.
                h, pv, p_isl, recip, raw = state
                r_mm = nc.tensor.matmul(
                    pv[0:DH, :],
                    ones_sb[DH : DH + 1, :],
                    recip[DH : DH + 1, :],
                    start=True,
                    stop=True,
                )
                if anchor is not None:
                    tile.add_dep_helper(
                        r_mm.ins, anchor.ins, sync=False,
                        reason="norm R after anchor QK (hide recip latency)",
                    )
                nc.vector.tensor_mul(
                    attn_sb[h][0:DH, p_isl], raw[:], pv[0:DH, :]
                )

            pending = None
            for it in range(NIT):
                isl = ts(it, 512)
                for hp in range(CCH):
                    for h in (2 * hp, 2 * hp + 1):
                        pv = ap.tile([DH + 1, 512], F32, tag="pv", bufs=2, name="pv")
                        for gi, g in enumerate(EXP_GROUPS):
                            sc = ap.tile([128, 512 * len(g)], F32, tag="sc",
                                         bufs=2, name="sc")
                            last_qk = None
                            for k, jc in enumerate(g):
                                last_qk = nc.tensor.matmul(
                                    sc[:, ts(k, 512)],
                                    kT_sb[hp][:, ts(jc, 128)],
                                    qTz_sb[hp][h % 2][:, isl],
                                    start=True,
                                    stop=True,
                                )
                            pt = pt_pool.tile([128, 512 * len(g)], BF16, tag="pt",
                                              name="pt")
                            nc.scalar.activation(
                                pt[:], sc[:], mybir.ActivationFunctionType.Exp,
                                bias=0.0, scale=SCALE,
                            )
                            if pending is not None and gi == 3:
                                norm_pe(pending, last_qk)
                                pending = None
                            for k, jc in enumerate(g):
                                nc.tensor.matmul(
                                    pv[:],
                                    v_sb[jc][:, h * VST : (h + 1) * VST],
                                    pt[:, ts(k, 512)],
                                    start=(jc == 0),
                                    stop=(jc == NJC - 1),
                                )
                        pending = norm_dve(h, pv, isl)
            norm_pe(pending, None)

        # ---------------- output projection + bias/residual ----------------
        with tc.tile_pool(name="op", bufs=1, space="PSUM") as op, \
             tc.tile_pool(name="ob", bufs=3) as ob:
            for ec in range(CCH):
                for it in range(NIT):
                    ps = op.tile([128, 512], F32, tag="yo", bufs=2, name="yo")
                    for h in range(H):
                        nc.tensor.matmul(
                            ps[:],
                            wo_sb[h][:, ts(ec, 128)],
                            attn_sb[h][:, ts(it, 512)],
                            start=(h == 0),
                            stop=(h == H - 1),
                        )
                    rt = ob.tile([128, 512], F32, tag="rt", name="rt")
                    nc.sync.dma_start(rt[:], res[ts(ec, 128), ts(it, 512)])
                    ot = ob.tile([128, 512], F32, tag="ot", name="ot")
                    nc.vector.tensor_add(ot[:], ps[:], rt[:])
                    nc.sync.dma_start(out[ts(ec, 128), ts(it, 512)], ot[:])

    _spill_matmul_waits(nc)
    return nc


# walrus embedded-sync-wait capacity per BIR opcode.  Matmult holds a
# single wait; excess waits hoist onto the paired Ldweights (in-order
# issue on PE makes that equivalent).  Other compute ops spill onto
# EventSemaphore carrier instructions inserted just before them on the
# same engine.  DMACopy / Drain / EventSemaphore handle many waits
# natively (bacc emits such itself) and are left alone.
_WAIT_CAPS = {
    "InstMatmult": 1,
    "InstLdweights": 1,
    "InstActivation": 1,
    "InstReciprocal": 1,
    "InstTensorTensor": 1,
    "InstTensorCopy": 1,
    "InstTensorScalarPtr": 1,
    "InstTensorReduce": 1,
    "InstMemset": 1,
    "InstDMACopy": 1,
    "InstDrain": 1,
    "InstCustomDveAnt": 1,
}
_ES_CAP = 2  # waits per EventSemaphore carrier (walrus: <=2 waits, <=1 update)


def _spill_matmul_waits(nc: bass.Bass) -> None:
    spill_id = [0]

    def carriers(excess, engine):
        out = []
        for i in range(0, len(excess), _ES_CAP):
            es = mybir.InstEventSemaphore(
                name=f"wait-spill-{spill_id[0]}", ins=[], outs=[]
            )
            spill_id[0] += 1
            es.engine = engine
            es.sync_info = mybir.SyncInfo(
                on_wait=excess[i : i + _ES_CAP], on_update=[]
            )
            out.append(es)
        return out

    for f in nc.m.functions:
        for blk in f.blocks:
            insts = blk.instructions
            i = 0
            while i < len(insts):
                inst = insts[i]
                tn = type(inst).__name__
                cap = _WAIT_CAPS.get(tn)
                si = inst.sync_info
                if cap is None or si is None or len(si.on_wait) <= cap:
                    i += 1
                    continue
                w = list(si.on_wait)
                if tn == "InstMatmult" and cap == 1:
                    # Keep the latest-satisfied dependency (the ACT-produced
                    # operand, e.g. probs from exp) embedded on the matmul and
                    # hoist early ones onto the Ldweights: a wait on the LDW
                    # blocks its background prefetch and serializes ~50ns of
                    # weight-load into every PV matmul.
                    acts = [x for x in w if "Activation" in (x.ant_name or "")]
                    if acts:
                        keep = [acts[-1]]
                        excess = [x for x in w if x is not acts[-1]]
                    else:
                        keep, excess = w[-cap:], w[:-cap]
                else:
                    keep, excess = w[-cap:], w[:-cap]
                prev = insts[i - 1] if i > 0 else None
                if (
                    tn == "InstMatmult"
                    and prev is not None
                    and type(prev).__name__ == "InstLdweights"
                    and len(((prev.sync_info and prev.sync_info.on_wait) or []))
                    + len(excess) <= 1
                ):
                    psi = prev.sync_info
                    pw = list(psi.on_wait) if psi is not None else []
                    pu = list(psi.on_update) if psi is not None else []
                    prev.sync_info = mybir.SyncInfo(on_wait=pw + excess, on_update=pu)
                else:
                    new = carriers(excess, inst.engine)
                    insts[i:i] = new
                    i += len(new)
                inst.sync_info = mybir.SyncInfo(
                    on_wait=keep, on_update=list(si.on_update)
                )
                i += 1


_CACHED_NC = None


def get_nc() -> bass.Bass:
    global _CACHED_NC
    if _CACHED_NC is None:
        _CACHED_NC = build_nc()
    return _CACHED_NC


def make_in_maps(hidden_states, Wq, Wk, Wv, Wo, b_out):
    hs = np.asarray(hidden_states, dtype=np.float32)
    bf = ml_dtypes.bfloat16
    wqT = np.ascontiguousarray(np.asarray(Wq, np.float32).T).astype(bf)
    wkT = np.ascontiguousarray(np.asarray(Wk, np.float32).T).astype(bf)
    wvT = np.ascontiguousarray(np.asarray(Wv, np.float32).T).astype(bf)
    woT = np.ascontiguousarray(np.asarray(Wo, np.float32).T).astype(bf)
    bias = np.asarray(b_out, np.float32).reshape(C, 1)
    in_maps = []
    for c in range(NCORES):
        b, g = divmod(c, GROUP)
        i0 = g * SQ
        hsTb = hs[b].T  # [C, S]
        in_maps.append(
            {
                "hsT": np.ascontiguousarray(np.roll(hsTb, -i0, axis=1)).astype(bf),
                "res": np.ascontiguousarray(hsTb[:, i0 : i0 + SQ]) + bias,
                "wqT": wqT,
                "wkT": wkT,
                "wvT": wvT,
                "woT": woT,
            }
        )
    return in_maps


def assemble(results) -> np.ndarray:
    y = np.empty((B, S, C), np.float32)
    for c in range(NCORES):
        b, g = divmod(c, GROUP)
        i0 = g * SQ
        y[b, i0 : i0 + SQ, :] = np.asarray(results[c]["out"], np.float32).T
    return y


def kernel(**inputs) -> np.ndarray:
    from concourse.bass_utils import run_bass_kernel_spmd

    nc = get_nc()
    in_maps = make_in_maps(**inputs)
    res = run_bass_kernel_spmd(nc, in_maps, list(range(NCORES)))
    return assemble(res.results)


if __name__ == "__main__":
    import reference

    inputs = {k: np.asarray(v) for k, v in reference.setup_inputs().items()}
    got = kernel(**inputs)
    want = np.asarray(reference.reference(**inputs))
    err = np.linalg.norm(got - want) / np.linalg.norm(want)
    print("Relative error:", err)
